# revision 24
# baseline (speedup 1.0000x reference)
"""ConcatCritic MLP on 8 Trainium2 NeuronCores.

Computes out[a, b] = f(concat(x[a], y[b])) for a tiny 4-layer MLP
(256->128->64->8->1 with ReLU), i.e. a [1024, 1024] score matrix.

Sharding (per spec hint): x's batch dim across the 8 cores (128 rows each);
y and the weights replicated. Each core computes a [128, 1024] output block.

Dataflow per core (feature-on-partition layout):
  - Split layer 1: concat(x,y) @ W1 = x @ W1[:128] + y @ W1[128:].
    xab[f, i] = (W1a^T @ x^T)[f, i] + b1[f]   (128 x 128, fp32)
    ybT[f, j] = (W1b^T @ y^T)[f, j]           (128 x 1024, bf16)
  - h1_i = relu(ybT + xab[:, i])  -- one DVE tensor_scalar per i (bf16, 4x)
  - L2: PE matmul, stationary W2 [128, 64]; even i -> PSUM rows 0:64
    (tile_position (0,0)), odd i -> rows 64:128 ((0,64)); one PSUM tile
    holds a pair of i's -> relu(+b2) evacuates [128, 1024] at once.
  - L3: stationary [128, 64] zero-padded 16-col strips (variant v for pair
    p = 4t+v) accumulate 4 pairs into each 64-row PSUM half; 8 pairs fill a
    dense [128, 1024] "h3pack" (16 i's) -> relu(+b3) evacuates at once.
  - L4: stationary [128, 64] with one W4 entry per (row-block, i) strip;
    8 groups accumulate into one [128, 1024] PSUM = the core's full output
    block (+b4 on evacuation).

Schedule (v2):
  - xT / yT arrive pre-cast to bf16 from the host (no on-device casts);
    inputs are split into three DRAM tensors by urgency (xT/yT/W1 first,
    W2/W3P/W4P second, fp32 biases last) so the L1 matmuls start as soon
    as possible after the fixed framework preamble.
  - Dummy warmup matmuls on never-written SBUF keep the PE busy during the
    input DMA so the HAM clock-gate is released (1.2 -> 2.4 GHz) before the
    real L2 stream begins, and the PE never idles >3.4us afterwards.
  - L3 of pair p is emitted after L2 of pair p+2, so the h2 evacuation
    (~1.2us on ScalarE/DVE) is covered by ~1.7us of L2 streaming and never
    stalls the PE.
  - h2 evacuations: ScalarE takes 7 of 8 pairs per group, DVE (which also
    produces every h1) takes the p==4 pair. h3 and output evacuations are
    split in half across both engines to halve their latency.
  - L4 of group g is deferred into the p==1/p==5 slots of later pairs; the
    output-row halves are evacuated/DMA'd as soon as their 4 groups of L4
    accumulation complete (rows 0:64 mid-kernel, rows 64:128 in the tail).

All matmuls are bf16 (1 cycle/col) in 128x64 column-tiling mode
(tile_positions (0,0)/(0,64) only). PSUM accumulation is fp32.
PSUM budget: ps2 2x[128,1024] + ps3 1x[128,1024] + pso 1x[128,1024]
= 8 banks exactly.
"""

import numpy as np
import ml_dtypes

import concourse.bass as bass
import concourse.bacc as bacc
import concourse.mybir as mybir
import concourse.tile as tile
from concourse.bass_utils import run_bass_kernel_spmd

BF16 = ml_dtypes.bfloat16
N_CORES = 8
B = 1024
D = 128
NI = B // N_CORES  # 128 rows of x per core
J = B              # full y batch per core
JC = 512           # matmul free-dim chunk (one PSUM bank)
JH = 512           # evac half-chunk

# bf16 input A (most urgent: weights + xT for the xab chain)
A_XT = 0             # [128, 128] x^T shard
A_W1A = A_XT + NI    # [128, 128]
A_W1B = A_W1A + D    # [128, 128]
A_TOT = A_W1B + D

# bf16 input Y (yT, second DMA)
Y_TOT = J

# bf16 input B (weights for L2..L4)
B_W2 = 0             # [128, 64]
B_W3P = B_W2 + 64    # [128, 4*64]
B_W4P = B_W3P + 256  # [128, 4*64]
B_TOT = B_W4P + 256

# fp32 input (biases)
F_B1 = 0
F_B2 = 1
F_B3 = 2
F_B4 = 3
F_TOT = 4

LOOKAHEAD = 4
NPAIR = 64
# GpSimd tensor_scalar measured ~15us per [128,1024] tile (software Q7
# implementation, ~30x slower than DVE) — do not offload h1 there.
GPSIMD_H1 = ()

_CACHE = {}


def _i_local_of_row(r):
    # h3pack row r -> which of the group's 16 i's it holds
    t, v, b = r // 64, (r % 64) // 16, (r % 16) // 8
    return 2 * (4 * t + v) + b


def _build_packed_weights(W3, W4):
    W3P = np.zeros((4, 128, 64), np.float32)
    for v in range(4):
        W3P[v, 0:64, 16 * v : 16 * v + 8] = W3
        W3P[v, 64:128, 16 * v + 8 : 16 * v + 16] = W3
    W4P = np.zeros((4, 128, 64), np.float32)
    for v4 in range(4):
        for r in range(128):
            c = 16 * v4 + _i_local_of_row(r)
            W4P[v4, r, c] = W4[r % 8, 0]
    return W3P, W4P


def _build_bass():
    nc = bacc.Bacc("TRN2", target_bir_lowering=False)
    f32 = mybir.dt.float32
    bf16 = mybir.dt.bfloat16

    bfa = nc.dram_tensor("bfa", [D, A_TOT], bf16, kind="ExternalInput")
    bfy = nc.dram_tensor("bfy", [D, Y_TOT], bf16, kind="ExternalInput")
    bfb = nc.dram_tensor("bfb", [D, B_TOT], bf16, kind="ExternalInput")
    f32in = nc.dram_tensor("f32in", [D, F_TOT], f32, kind="ExternalInput")
    outd = nc.dram_tensor("out", [NI, J], f32, kind="ExternalOutput")

    RELU = mybir.ActivationFunctionType.Relu
    IDENT = mybir.ActivationFunctionType.Identity
    ADD = mybir.AluOpType.add
    MAX = mybir.AluOpType.max

    with tile.TileContext(nc) as tc:
        with (
            tc.tile_pool(name="const", bufs=1) as cpool,
            tc.tile_pool(name="work", bufs=16) as work,
            tc.tile_pool(name="h2p", bufs=6) as h2pool,
            tc.tile_pool(name="h3p", bufs=3) as h3pool,
            tc.tile_pool(name="ps2", bufs=2, space="PSUM") as ps2,
            tc.tile_pool(name="ps3", bufs=1, space="PSUM") as ps3,
            tc.tile_pool(name="pso", bufs=1, space="PSUM") as pso,
        ):
            hina = cpool.tile([D, A_TOT], bf16)
            hiny = cpool.tile([D, Y_TOT], bf16)
            hinb = cpool.tile([D, B_TOT], bf16)
            fin = cpool.tile([D, F_TOT], f32)
            dummy = cpool.tile([D, JC], bf16)  # memset: warmup operand
            scr = cpool.tile([D, 1], bf16)     # ACT table preload target
            xab = cpool.tile([D, NI], f32)
            ybT = cpool.tile([D, J], bf16)
            out_sb = cpool.tile([NI, J], f32)

            # ---- PE warmup: garbage matmuls into the ps3 slot keep the
            # HAM activity window busy while the input DMA lands. start=True
            # overwrites; group 0's L3 (WAW on the ps3 slot) runs much later.
            nc.gpsimd.memset(dummy[:], 0.0)
            # trigger the one-time ACT_TABLE_LOAD (~2.7us) immediately so it
            # doesn't delay the first real evacuation
            nc.scalar.activation(scr[:], dummy[:, 0:1], RELU)
            wu_ps = ps3.tile([D, J], mybir.dt.float32, tag="ps3")
            for w in range(5):
                nc.tensor.matmul(
                    wu_ps[0:64, 0:JC], dummy[:, 0:64], dummy[:],
                    tile_position=(0, 0), skip_group_check=True,
                )

            nc.sync.dma_start(hina[:], bfa[:])
            nc.sync.dma_start(hiny[:], bfy[:])
            nc.sync.dma_start(fin[:], f32in[:])
            nc.sync.dma_start(hinb[:], bfb[:])

            W2_sb = hinb[:, B_W2 : B_W2 + 64]
            b1_sb = fin[:, F_B1 : F_B1 + 1]
            b2_sb = fin[:, F_B2 : F_B2 + 1]
            b3_sb = fin[:, F_B3 : F_B3 + 1]
            b4_sb = fin[:, F_B4 : F_B4 + 1]

            # ---- precompute: xab = W1a^T @ xT + b1 ; ybT = W1b^T @ yT ----
            xa_ps = ps2.tile([D, J], mybir.dt.float32, tag="ps2")
            for c in range(2):
                nc.tensor.matmul(
                    xa_ps[64 * c : 64 * c + 64, :NI],
                    hina[:, A_W1A + 64 * c : A_W1A + 64 * c + 64],
                    hina[:, A_XT : A_XT + NI],
                    tile_position=(0, 64 * c),
                )
            # keep the PE busy until yT lands (kicked after bfa)
            for w in range(2):
                nc.tensor.matmul(
                    wu_ps[0:64, 0:JC], dummy[:, 0:64], dummy[:],
                    tile_position=(0, 0), skip_group_check=True,
                )
            nc.scalar.activation(xab[:], xa_ps[:, :NI], IDENT, bias=b1_sb)

            yb_ps = ps2.tile([D, J], mybir.dt.float32, tag="ps2")
            for c in range(2):
                for jc in range(2):
                    nc.tensor.matmul(
                        yb_ps[64 * c : 64 * c + 64, JC * jc : JC * jc + JC],
                        hina[:, A_W1B + 64 * c : A_W1B + 64 * c + 64],
                        hiny[:, JC * jc : JC * jc + JC],
                        tile_position=(0, 64 * c),
                    )
            # evac split across both engines to halve latency
            nc.scalar.activation(ybT[:, 0:JH], yb_ps[:, 0:JH], IDENT)
            nc.vector.tensor_scalar(
                ybT[:, JH:J], yb_ps[:, JH:J], 0.0, None, ADD
            )

            # second warmup burst: covers the gap between the L1 matmuls
            # and the first L2 while ybT/h1 are being produced
            for w in range(4):
                nc.tensor.matmul(
                    wu_ps[0:64, 0:JC], dummy[:, 0:64], dummy[:],
                    tile_position=(0, 0), skip_group_check=True,
                )

            # ---- main loop (software-pipelined emission) ----
            out_ps = pso.tile([D, J], mybir.dt.float32)

            def x_col(gp):
                g, p = gp // 8, gp % 8
                return 64 * (g // 4) + 16 * (g % 4) + 2 * p

            def emit_h1(gp):
                # one h1 of GPSIMD_H1 pairs per group is produced on the
                # otherwise-idle GpSimd engine (slower, but free capacity;
                # it is emitted LOOKAHEAD pairs early so latency hides)
                h1e = work.tile([D, J], bf16, tag="h1")
                h1o = work.tile([D, J], bf16, tag="h1")
                ie = x_col(gp)
                g, p = gp // 8, gp % 8
                on_gps = p in GPSIMD_H1 and g > 0
                if on_gps:
                    nc.gpsimd.tensor_scalar(
                        h1o[:], ybT[:], xab[:, ie + 1 : ie + 2], 0.0, ADD, MAX
                    )
                nc.vector.tensor_scalar(
                    h1e[:], ybT[:], xab[:, ie : ie + 1], 0.0, ADD, MAX
                )
                if not on_gps:
                    nc.vector.tensor_scalar(
                        h1o[:], ybT[:], xab[:, ie + 1 : ie + 2], 0.0, ADD, MAX
                    )
                return h1e, h1o

            def mm_l2(ps2_t, h1, jc, odd):
                jsl = slice(JC * jc, JC * jc + JC)
                nc.tensor.matmul(
                    ps2_t[64 * odd : 64 * odd + 64, jsl], W2_sb, h1[:, jsl],
                    tile_position=(0, 64 * odd),
                )

            def mm_l3(gp, jc):
                p = gp % 8
                t, v = p // 4, p % 4
                jsl = slice(JC * jc, JC * jc + JC)
                nc.tensor.matmul(
                    ps3_t[64 * t : 64 * t + 64, jsl],
                    hinb[:, B_W3P + 64 * v : B_W3P + 64 * v + 64],
                    l3q[gp][:, jsl],
                    tile_position=(0, 64 * t),
                    start=(v == 0),
                    stop=(v == 3),
                )

            def mm_l4(g, h3pack, jc):
                t4, v4 = g // 4, g % 4
                jsl = slice(JC * jc, JC * jc + JC)
                nc.tensor.matmul(
                    out_ps[64 * t4 : 64 * t4 + 64, jsl],
                    hinb[:, B_W4P + 64 * v4 : B_W4P + 64 * v4 + 64],
                    h3pack[:, jsl],
                    tile_position=(0, 64 * t4),
                    start=(v4 == 0),
                    stop=(v4 == 3),
                )

            def emit_evac(gp, ps2_t, split=False):
                h2pack = h2pool.tile([D, J], bf16, tag="h2")
                if split:
                    # latency-critical (kernel tail): halve it across engines
                    nc.scalar.activation(
                        h2pack[:, 0:JH], ps2_t[:, 0:JH], RELU, bias=b2_sb
                    )
                    nc.vector.tensor_scalar(
                        h2pack[:, JH:J], ps2_t[:, JH:J], b2_sb, 0.0, ADD, MAX
                    )
                else:
                    nc.scalar.activation(
                        h2pack[:], ps2_t[:], RELU, bias=b2_sb
                    )
                return h2pack

            def emit_h3_evac(g):
                # split across both engines (asymmetric: DVE is lighter)
                h3pack = h3pool.tile([D, J], bf16, tag="h3")
                HS = 320
                nc.scalar.activation(
                    h3pack[:, 0:HS], ps3_t[:, 0:HS], RELU, bias=b3_sb
                )
                nc.vector.tensor_scalar(
                    h3pack[:, HS:J], ps3_t[:, HS:J], b3_sb, 0.0, ADD, MAX
                )
                return h3pack

            def emit_out_half(t4):
                rsl = slice(64 * t4, 64 * t4 + 64)
                nc.scalar.activation(
                    out_sb[rsl, 0:JH], out_ps[rsl, 0:JH], IDENT,
                    bias=b4_sb[rsl, :],
                )
                nc.sync.dma_start(outd[rsl, 0:JH], out_sb[rsl, 0:JH])
                nc.vector.tensor_scalar(
                    out_sb[rsl, JH:J], out_ps[rsl, JH:J],
                    b4_sb[rsl, :], 0.0, ADD,
                )
                nc.sync.dma_start(outd[rsl, JH:J], out_sb[rsl, JH:J])

            h1q = {gp: emit_h1(gp) for gp in range(LOOKAHEAD)}
            l3q = {}        # gp -> h2pack awaiting L3 (deferred by 2 pairs)
            pend_l4 = []    # [(g, h3pack)] awaiting L4 emission
            ps3_t = None

            for gp in range(NPAIR):
                g, p = gp // 8, gp % 8
                if p == 2:
                    # first L3 of group g lands now; claim its psum tile
                    ps3_t = ps3.tile([D, J], mybir.dt.float32, tag="ps3")
                if gp + LOOKAHEAD < NPAIR:
                    h1q[gp + LOOKAHEAD] = emit_h1(gp + LOOKAHEAD)
                h1e, h1o = h1q.pop(gp)
                ps2_t = ps2.tile([D, J], mybir.dt.float32, tag="ps2")
                dq = gp - 2 if gp >= 2 else None
                l4 = None
                if pend_l4 and (
                    (p == 1 and pend_l4[0][0] < 4)
                    or (p == 3 and pend_l4[0][0] >= 4)
                ):
                    l4 = pend_l4.pop(0)
                # ---- interleaved MM emission: strict h0/h64 alternation so
                # the two stationary col-groups stream concurrently ----
                if dq is None:
                    for jc in range(2):
                        mm_l2(ps2_t, h1e, jc, 0)
                        mm_l2(ps2_t, h1o, jc, 1)
                else:
                    t_l3 = (dq % 8) // 4  # 0: L3 MMs are h0; 1: h64
                    if l4 is not None:
                        lg, lh = l4
                        if t_l3 == 1:  # L4 is h0 (lg<4): 4 h0 + 4 h64
                            mm_l2(ps2_t, h1e, 0, 0)
                            mm_l2(ps2_t, h1o, 0, 1)
                            mm_l4(lg, lh, 0)
                            mm_l3(dq, 0)
                            mm_l4(lg, lh, 1)
                            mm_l3(dq, 1)
                            mm_l2(ps2_t, h1e, 1, 0)
                            mm_l2(ps2_t, h1o, 1, 1)
                        else:          # L4 is h64 (lg>=4): 4 h64 + 4 h0
                            mm_l2(ps2_t, h1o, 0, 1)
                            mm_l2(ps2_t, h1e, 0, 0)
                            mm_l4(lg, lh, 0)
                            mm_l3(dq, 0)
                            mm_l4(lg, lh, 1)
                            mm_l3(dq, 1)
                            mm_l2(ps2_t, h1o, 1, 1)
                            mm_l2(ps2_t, h1e, 1, 0)
                    elif t_l3 == 0:    # h0-heavy slot: chain e0,L3a,L3b,e1
                        mm_l2(ps2_t, h1e, 0, 0)
                        mm_l2(ps2_t, h1o, 0, 1)
                        mm_l3(dq, 0)
                        mm_l2(ps2_t, h1o, 1, 1)
                        mm_l3(dq, 1)
                        mm_l2(ps2_t, h1e, 1, 0)
                    else:              # h64-heavy slot
                        mm_l2(ps2_t, h1o, 0, 1)
                        mm_l2(ps2_t, h1e, 0, 0)
                        mm_l3(dq, 0)
                        mm_l2(ps2_t, h1e, 1, 0)
                        mm_l3(dq, 1)
                        mm_l2(ps2_t, h1o, 1, 1)
                if dq is not None:
                    l3q.pop(dq)
                    if dq % 8 == 7:
                        h3pack = emit_h3_evac(dq // 8)
                        pend_l4.append((dq // 8, h3pack))
                l3q[gp] = emit_evac(gp, ps2_t, split=(gp >= 62))
                if g == 5 and p == 4:
                    emit_out_half(0)

            # ---- tail ----
            for dq in (62, 63):
                mm_l3(dq, 0)
                mm_l3(dq, 1)
                l3q.pop(dq)
            h3pack = emit_h3_evac(7)
            pend_l4.append((7, h3pack))
            while pend_l4:
                lg, lh = pend_l4.pop(0)
                mm_l4(lg, lh, 0)
                mm_l4(lg, lh, 1)
            emit_out_half(1)

    nc.compile()
    return nc


def _get_compiled():
    if "nc" not in _CACHE:
        _CACHE["nc"] = _build_bass()
    return _CACHE["nc"]


def _prep_in_maps(x, y, W1, b1, W2, b2, W3, b3, W4, b4):
    d = x.shape[1]
    W1a = W1[:d]
    W1b = W1[d:]
    W3P, W4P = _build_packed_weights(W3, W4)

    f32pack = np.empty((D, F_TOT), np.float32)
    f32pack[:, F_B1] = b1
    f32pack[:, F_B2] = np.concatenate([b2, b2])
    f32pack[:, F_B3] = np.tile(b3, 16)
    f32pack[:, F_B4] = b4[0]
    f32pack = np.ascontiguousarray(f32pack)

    bpack = np.empty((D, B_TOT), BF16)
    bpack[:, B_W2 : B_W2 + 64] = W2.astype(BF16)
    bpack[:, B_W3P : B_W3P + 256] = (
        W3P.transpose(1, 0, 2).reshape(D, 256).astype(BF16)
    )
    bpack[:, B_W4P : B_W4P + 256] = (
        W4P.transpose(1, 0, 2).reshape(D, 256).astype(BF16)
    )
    bpack = np.ascontiguousarray(bpack)

    apack = np.empty((D, A_TOT), BF16)
    apack[:, A_W1A : A_W1A + D] = W1a.astype(BF16)
    apack[:, A_W1B : A_W1B + D] = W1b.astype(BF16)
    ypack = np.ascontiguousarray(y.T.astype(BF16))

    in_maps = []
    for c in range(N_CORES):
        ap = apack.copy()
        ap[:, A_XT : A_XT + NI] = x[c * NI : (c + 1) * NI].T.astype(BF16)
        in_maps.append(
            {"bfa": ap, "bfy": ypack, "bfb": bpack, "f32in": f32pack}
        )
    return in_maps


def run(x, y, W1, b1, W2, b2, W3, b3, W4, b4, **spmd_kwargs):
    """Run the kernel, returning (output, BassKernelResults)."""
    args = [np.asarray(a, np.float32) for a in
            (x, y, W1, b1, W2, b2, W3, b3, W4, b4)]
    in_maps = _prep_in_maps(*args)
    nc = _get_compiled()
    res = run_bass_kernel_spmd(nc, in_maps, list(range(N_CORES)), **spmd_kwargs)
    out = np.concatenate([np.asarray(r["out"]) for r in res.results], axis=0)
    return out.astype(np.float32), res


def kernel(x, y, W1, b1, W2, b2, W3, b3, W4, b4):
    out, _ = run(x, y, W1, b1, W2, b2, W3, b3, W4, b4)
    return out


# revision 25
# speedup vs baseline: 1.0325x; 1.0325x over previous
"""ConcatCritic MLP on 8 Trainium2 NeuronCores.

Computes out[a, b] = f(concat(x[a], y[b])) for a tiny 4-layer MLP
(256->128->64->8->1 with ReLU), i.e. a [1024, 1024] score matrix.

Sharding (per spec hint): x's batch dim across the 8 cores (128 rows each);
y and the weights replicated. Each core computes a [128, 1024] output block.

Dataflow per core (feature-on-partition layout):
  - Split layer 1: concat(x,y) @ W1 = x @ W1[:128] + y @ W1[128:].
    xab[f, i] = (W1a^T @ x^T)[f, i] + b1[f]   (128 x 128, fp32)
    ybT[f, j] = (W1b^T @ y^T)[f, j]           (128 x 1024, bf16)
  - h1_i = relu(ybT + xab[:, i])  -- one DVE tensor_scalar per i (bf16, 4x)
  - L2: PE matmul, stationary W2 [128, 64]; even i -> PSUM rows 0:64
    (tile_position (0,0)), odd i -> rows 64:128 ((0,64)); one PSUM tile
    holds a pair of i's -> relu(+b2) evacuates [128, 1024] at once.
  - L3: stationary [128, 64] zero-padded 16-col strips (variant v for pair
    p = 4t+v) accumulate 4 pairs into each 64-row PSUM half; 8 pairs fill a
    dense [128, 1024] "h3pack" (16 i's) -> relu(+b3) evacuates at once.
  - L4: stationary [128, 64] with one W4 entry per (row-block, i) strip;
    8 groups accumulate into one [128, 1024] PSUM = the core's full output
    block (+b4 on evacuation).

Schedule (v2):
  - xT / yT arrive pre-cast to bf16 from the host (no on-device casts);
    inputs are split into three DRAM tensors by urgency (xT/yT/W1 first,
    W2/W3P/W4P second, fp32 biases last) so the L1 matmuls start as soon
    as possible after the fixed framework preamble.
  - Dummy warmup matmuls on never-written SBUF keep the PE busy during the
    input DMA so the HAM clock-gate is released (1.2 -> 2.4 GHz) before the
    real L2 stream begins, and the PE never idles >3.4us afterwards.
  - L3 of pair p is emitted after L2 of pair p+2, so the h2 evacuation
    (~1.2us on ScalarE/DVE) is covered by ~1.7us of L2 streaming and never
    stalls the PE.
  - h2 evacuations: ScalarE takes 7 of 8 pairs per group, DVE (which also
    produces every h1) takes the p==4 pair. h3 and output evacuations are
    split in half across both engines to halve their latency.
  - L4 of group g is deferred into the p==1/p==5 slots of later pairs; the
    output-row halves are evacuated/DMA'd as soon as their 4 groups of L4
    accumulation complete (rows 0:64 mid-kernel, rows 64:128 in the tail).

All matmuls are bf16 (1 cycle/col) in 128x64 column-tiling mode
(tile_positions (0,0)/(0,64) only). PSUM accumulation is fp32.
PSUM budget: ps2 2x[128,1024] + ps3 1x[128,1024] + pso 1x[128,1024]
= 8 banks exactly.
"""

import numpy as np
import ml_dtypes

import concourse.bass as bass
import concourse.bacc as bacc
import concourse.mybir as mybir
import concourse.tile as tile
from concourse.bass_utils import run_bass_kernel_spmd

BF16 = ml_dtypes.bfloat16
N_CORES = 8
B = 1024
D = 128
NI = B // N_CORES  # 128 rows of x per core
J = B              # full y batch per core
JC = 512           # matmul free-dim chunk (one PSUM bank)
JH = 512           # evac half-chunk

# bf16 input A (most urgent: weights + xT for the xab chain)
A_XT = 0             # [128, 128] x^T shard
A_W1A = A_XT + NI    # [128, 128]
A_W1B = A_W1A + D    # [128, 128]
A_TOT = A_W1B + D

# bf16 input Y (yT, second DMA)
Y_TOT = J

# bf16 input B (weights for L2..L4)
B_W2 = 0             # [128, 64]
B_W3P = B_W2 + 64    # [128, 4*64]
B_W4P = B_W3P + 256  # [128, 4*64]
B_TOT = B_W4P + 256

# fp32 input (biases)
F_B1 = 0
F_B2 = 1
F_B3 = 2
F_B4 = 3
F_TOT = 4

LOOKAHEAD = 4
NPAIR = 64
# GpSimd tensor_scalar measured ~15us per [128,1024] tile (software Q7
# implementation, ~30x slower than DVE) — do not offload h1 there.
GPSIMD_H1 = ()

_CACHE = {}


def _i_local_of_row(r):
    # h3pack row r -> which of the group's 16 i's it holds
    t, v, b = r // 64, (r % 64) // 16, (r % 16) // 8
    return 2 * (4 * t + v) + b


def _build_packed_weights(W3, W4):
    W3P = np.zeros((4, 128, 64), np.float32)
    for v in range(4):
        W3P[v, 0:64, 16 * v : 16 * v + 8] = W3
        W3P[v, 64:128, 16 * v + 8 : 16 * v + 16] = W3
    W4P = np.zeros((4, 128, 64), np.float32)
    for v4 in range(4):
        for r in range(128):
            c = 16 * v4 + _i_local_of_row(r)
            W4P[v4, r, c] = W4[r % 8, 0]
    return W3P, W4P


def _build_bass():
    nc = bacc.Bacc("TRN2", target_bir_lowering=False)
    f32 = mybir.dt.float32
    bf16 = mybir.dt.bfloat16

    bfa = nc.dram_tensor("bfa", [D, A_TOT], bf16, kind="ExternalInput")
    bfy = nc.dram_tensor("bfy", [D, Y_TOT], bf16, kind="ExternalInput")
    bfb = nc.dram_tensor("bfb", [D, B_TOT], bf16, kind="ExternalInput")
    f32in = nc.dram_tensor("f32in", [D, F_TOT], f32, kind="ExternalInput")
    outd = nc.dram_tensor("out", [NI, J], f32, kind="ExternalOutput")

    RELU = mybir.ActivationFunctionType.Relu
    IDENT = mybir.ActivationFunctionType.Identity
    ADD = mybir.AluOpType.add
    MAX = mybir.AluOpType.max

    with tile.TileContext(nc) as tc:
        with (
            tc.tile_pool(name="const", bufs=1) as cpool,
            tc.tile_pool(name="work", bufs=16) as work,
            tc.tile_pool(name="h2p", bufs=6) as h2pool,
            tc.tile_pool(name="h3p", bufs=3) as h3pool,
            tc.tile_pool(name="ps2", bufs=2, space="PSUM") as ps2,
            tc.tile_pool(name="ps3", bufs=1, space="PSUM") as ps3,
            tc.tile_pool(name="pso", bufs=1, space="PSUM") as pso,
        ):
            hina = cpool.tile([D, A_TOT], bf16)
            hiny = cpool.tile([D, Y_TOT], bf16)
            hinb = cpool.tile([D, B_TOT], bf16)
            fin = cpool.tile([D, F_TOT], f32)
            dummy = cpool.tile([D, JC], bf16)  # memset: warmup operand
            scr = cpool.tile([D, 1], bf16)     # ACT table preload target
            xab = cpool.tile([D, NI], f32)
            ybT = cpool.tile([D, J], bf16)
            out_sb = cpool.tile([NI, J], f32)

            # ---- PE warmup: garbage matmuls into the ps3 slot keep the
            # HAM activity window busy while the input DMA lands. start=True
            # overwrites; group 0's L3 (WAW on the ps3 slot) runs much later.
            nc.gpsimd.memset(dummy[:], 0.0)
            # trigger the one-time ACT_TABLE_LOAD (~2.7us) immediately so it
            # doesn't delay the first real evacuation
            nc.scalar.activation(scr[:], dummy[:, 0:1], RELU)
            wu_ps = ps3.tile([D, J], mybir.dt.float32, tag="ps3")
            for w in range(5):
                nc.tensor.matmul(
                    wu_ps[0:64, 0:JC], dummy[:, 0:64], dummy[:],
                    tile_position=(0, 0), skip_group_check=True,
                )

            nc.sync.dma_start(hina[:], bfa[:])
            nc.sync.dma_start(hiny[:], bfy[:])
            nc.sync.dma_start(fin[:], f32in[:])
            nc.sync.dma_start(hinb[:], bfb[:])

            W2_sb = hinb[:, B_W2 : B_W2 + 64]
            b1_sb = fin[:, F_B1 : F_B1 + 1]
            b2_sb = fin[:, F_B2 : F_B2 + 1]
            b3_sb = fin[:, F_B3 : F_B3 + 1]
            b4_sb = fin[:, F_B4 : F_B4 + 1]

            # ---- precompute: xab = W1a^T @ xT + b1 ; ybT = W1b^T @ yT ----
            xa_ps = ps2.tile([D, J], mybir.dt.float32, tag="ps2")
            for c in range(2):
                nc.tensor.matmul(
                    xa_ps[64 * c : 64 * c + 64, :NI],
                    hina[:, A_W1A + 64 * c : A_W1A + 64 * c + 64],
                    hina[:, A_XT : A_XT + NI],
                    tile_position=(0, 64 * c),
                )
            # keep the PE busy until yT lands (kicked after bfa)
            for w in range(2):
                nc.tensor.matmul(
                    wu_ps[0:64, 0:JC], dummy[:, 0:64], dummy[:],
                    tile_position=(0, 0), skip_group_check=True,
                )
            nc.scalar.activation(xab[:], xa_ps[:, :NI], IDENT, bias=b1_sb)

            yb_ps = ps2.tile([D, J], mybir.dt.float32, tag="ps2")
            for c in range(2):
                for jc in range(2):
                    nc.tensor.matmul(
                        yb_ps[64 * c : 64 * c + 64, JC * jc : JC * jc + JC],
                        hina[:, A_W1B + 64 * c : A_W1B + 64 * c + 64],
                        hiny[:, JC * jc : JC * jc + JC],
                        tile_position=(0, 64 * c),
                    )
            # evac split across both engines to halve latency
            nc.scalar.activation(ybT[:, 0:JH], yb_ps[:, 0:JH], IDENT)
            nc.vector.tensor_scalar(
                ybT[:, JH:J], yb_ps[:, JH:J], 0.0, None, ADD
            )

            # second warmup burst: covers the gap between the L1 matmuls
            # and the first L2 while ybT/h1 are being produced
            for w in range(4):
                nc.tensor.matmul(
                    wu_ps[0:64, 0:JC], dummy[:, 0:64], dummy[:],
                    tile_position=(0, 0), skip_group_check=True,
                )

            # ---- main loop (software-pipelined emission) ----
            out_ps = pso.tile([D, J], mybir.dt.float32)

            def x_col(gp):
                g, p = gp // 8, gp % 8
                return 64 * (g // 4) + 16 * (g % 4) + 2 * p

            def emit_h1(gp):
                # one h1 of GPSIMD_H1 pairs per group is produced on the
                # otherwise-idle GpSimd engine (slower, but free capacity;
                # it is emitted LOOKAHEAD pairs early so latency hides)
                h1e = work.tile([D, J], bf16, tag="h1")
                h1o = work.tile([D, J], bf16, tag="h1")
                ie = x_col(gp)
                g, p = gp // 8, gp % 8
                on_gps = p in GPSIMD_H1 and g > 0
                if on_gps:
                    nc.gpsimd.tensor_scalar(
                        h1o[:], ybT[:], xab[:, ie + 1 : ie + 2], 0.0, ADD, MAX
                    )
                nc.vector.tensor_scalar(
                    h1e[:], ybT[:], xab[:, ie : ie + 1], 0.0, ADD, MAX
                )
                if not on_gps:
                    nc.vector.tensor_scalar(
                        h1o[:], ybT[:], xab[:, ie + 1 : ie + 2], 0.0, ADD, MAX
                    )
                return h1e, h1o

            def mm_l2(ps2_t, h1, jc, odd):
                jsl = slice(JC * jc, JC * jc + JC)
                nc.tensor.matmul(
                    ps2_t[64 * odd : 64 * odd + 64, jsl], W2_sb, h1[:, jsl],
                    tile_position=(0, 64 * odd),
                )

            def mm_l3(gp, jc):
                p = gp % 8
                t, v = p // 4, p % 4
                jsl = slice(JC * jc, JC * jc + JC)
                nc.tensor.matmul(
                    ps3_t[64 * t : 64 * t + 64, jsl],
                    hinb[:, B_W3P + 64 * v : B_W3P + 64 * v + 64],
                    l3q[gp][:, jsl],
                    tile_position=(0, 64 * t),
                    start=(v == 0),
                    stop=(v == 3),
                )

            def mm_l4(g, h3pack, jc):
                t4, v4 = g // 4, g % 4
                jsl = slice(JC * jc, JC * jc + JC)
                nc.tensor.matmul(
                    out_ps[64 * t4 : 64 * t4 + 64, jsl],
                    hinb[:, B_W4P + 64 * v4 : B_W4P + 64 * v4 + 64],
                    h3pack[:, jsl],
                    tile_position=(0, 64 * t4),
                    start=(v4 == 0),
                    stop=(v4 == 3),
                )

            def emit_evac(gp, ps2_t, split=False):
                h2pack = h2pool.tile([D, J], bf16, tag="h2")
                if split:
                    # latency-critical (kernel tail): halve it across engines
                    nc.scalar.activation(
                        h2pack[:, 0:JH], ps2_t[:, 0:JH], RELU, bias=b2_sb
                    )
                    nc.vector.tensor_scalar(
                        h2pack[:, JH:J], ps2_t[:, JH:J], b2_sb, 0.0, ADD, MAX
                    )
                else:
                    nc.scalar.activation(
                        h2pack[:], ps2_t[:], RELU, bias=b2_sb
                    )
                return h2pack

            def emit_h3_evac(g):
                # split across both engines (asymmetric: DVE is lighter)
                h3pack = h3pool.tile([D, J], bf16, tag="h3")
                HS = 320
                nc.scalar.activation(
                    h3pack[:, 0:HS], ps3_t[:, 0:HS], RELU, bias=b3_sb
                )
                nc.vector.tensor_scalar(
                    h3pack[:, HS:J], ps3_t[:, HS:J], b3_sb, 0.0, ADD, MAX
                )
                return h3pack

            def emit_out_half(t4):
                rsl = slice(64 * t4, 64 * t4 + 64)
                nc.scalar.activation(
                    out_sb[rsl, 0:JH], out_ps[rsl, 0:JH], IDENT,
                    bias=b4_sb[rsl, :],
                )
                nc.sync.dma_start(outd[rsl, 0:JH], out_sb[rsl, 0:JH])
                nc.vector.tensor_scalar(
                    out_sb[rsl, JH:J], out_ps[rsl, JH:J],
                    b4_sb[rsl, :], 0.0, ADD,
                )
                nc.sync.dma_start(outd[rsl, JH:J], out_sb[rsl, JH:J])

            h1q = {gp: emit_h1(gp) for gp in range(LOOKAHEAD)}
            l3q = {}        # gp -> h2pack awaiting L3 (deferred by 2 pairs)
            pend_l4 = []    # [(g, h3pack)] awaiting L4 emission
            ps3_t = None

            for gp in range(NPAIR):
                g, p = gp // 8, gp % 8
                if p == 2:
                    # first L3 of group g lands now; claim its psum tile
                    ps3_t = ps3.tile([D, J], mybir.dt.float32, tag="ps3")
                if gp + LOOKAHEAD < NPAIR:
                    h1q[gp + LOOKAHEAD] = emit_h1(gp + LOOKAHEAD)
                h1e, h1o = h1q.pop(gp)
                ps2_t = ps2.tile([D, J], mybir.dt.float32, tag="ps2")
                # L2 first (its completion releases the evacuation), then the
                # deferred L4/L3 which cover the previous pairs' evac latency
                for jc in range(2):
                    mm_l2(ps2_t, h1e, jc, 0)
                    mm_l2(ps2_t, h1o, jc, 1)
                if p in (1, 5) and pend_l4:
                    lg, lh = pend_l4.pop(0)
                    mm_l4(lg, lh, 0)
                    mm_l4(lg, lh, 1)
                dq = gp - 2
                if dq >= 0:
                    mm_l3(dq, 0)
                    mm_l3(dq, 1)
                    l3q.pop(dq)
                    if dq % 8 == 7:
                        h3pack = emit_h3_evac(dq // 8)
                        pend_l4.append((dq // 8, h3pack))
                l3q[gp] = emit_evac(gp, ps2_t, split=(p == 4 or gp >= 62))
                if g == 5 and p == 3:
                    emit_out_half(0)

            # ---- tail ----
            for dq in (62, 63):
                mm_l3(dq, 0)
                mm_l3(dq, 1)
                l3q.pop(dq)
            h3pack = emit_h3_evac(7)
            pend_l4.append((7, h3pack))
            while pend_l4:
                lg, lh = pend_l4.pop(0)
                mm_l4(lg, lh, 0)
                mm_l4(lg, lh, 1)
            emit_out_half(1)

    nc.compile()
    return nc


def _get_compiled():
    if "nc" not in _CACHE:
        _CACHE["nc"] = _build_bass()
    return _CACHE["nc"]


def _prep_in_maps(x, y, W1, b1, W2, b2, W3, b3, W4, b4):
    d = x.shape[1]
    W1a = W1[:d]
    W1b = W1[d:]
    W3P, W4P = _build_packed_weights(W3, W4)

    f32pack = np.empty((D, F_TOT), np.float32)
    f32pack[:, F_B1] = b1
    f32pack[:, F_B2] = np.concatenate([b2, b2])
    f32pack[:, F_B3] = np.tile(b3, 16)
    f32pack[:, F_B4] = b4[0]
    f32pack = np.ascontiguousarray(f32pack)

    bpack = np.empty((D, B_TOT), BF16)
    bpack[:, B_W2 : B_W2 + 64] = W2.astype(BF16)
    bpack[:, B_W3P : B_W3P + 256] = (
        W3P.transpose(1, 0, 2).reshape(D, 256).astype(BF16)
    )
    bpack[:, B_W4P : B_W4P + 256] = (
        W4P.transpose(1, 0, 2).reshape(D, 256).astype(BF16)
    )
    bpack = np.ascontiguousarray(bpack)

    apack = np.empty((D, A_TOT), BF16)
    apack[:, A_W1A : A_W1A + D] = W1a.astype(BF16)
    apack[:, A_W1B : A_W1B + D] = W1b.astype(BF16)
    ypack = np.ascontiguousarray(y.T.astype(BF16))

    in_maps = []
    for c in range(N_CORES):
        ap = apack.copy()
        ap[:, A_XT : A_XT + NI] = x[c * NI : (c + 1) * NI].T.astype(BF16)
        in_maps.append(
            {"bfa": ap, "bfy": ypack, "bfb": bpack, "f32in": f32pack}
        )
    return in_maps


def run(x, y, W1, b1, W2, b2, W3, b3, W4, b4, **spmd_kwargs):
    """Run the kernel, returning (output, BassKernelResults)."""
    args = [np.asarray(a, np.float32) for a in
            (x, y, W1, b1, W2, b2, W3, b3, W4, b4)]
    in_maps = _prep_in_maps(*args)
    nc = _get_compiled()
    res = run_bass_kernel_spmd(nc, in_maps, list(range(N_CORES)), **spmd_kwargs)
    out = np.concatenate([np.asarray(r["out"]) for r in res.results], axis=0)
    return out.astype(np.float32), res


def kernel(x, y, W1, b1, W2, b2, W3, b3, W4, b4):
    out, _ = run(x, y, W1, b1, W2, b2, W3, b3, W4, b4)
    return out
